# revision 1
# baseline (speedup 1.0000x reference)
"""CosineEmbeddingLoss-style kernel for Trainium2 (Bass/Tile), 8-core data parallel.

reference semantics (fp32):
    dot   = sum(x*y, -1); xx = sum(x*x, -1); yy = sum(y*y, -1)
    d     = dot / max(sqrt(xx*yy), EPS)
    per   = where(p == 1, 1 - d, max(0, d - MARGIN))
    loss  = sum(per)

Sharding: rows (N) split contiguously across 8 cores; each core returns its
(128,1) f32 partition partials; host sums them.

Per-core schedule: host interleaves x and y chunk-by-chunk into one DRAM
tensor so each chunk is a single dma_start (x and y of a chunk land together).
Chunks are DMA'd p-major (each SBUF partition holds s consecutive rows → large
contiguous HBM reads). Per 128-row group: dot via DVE scalar_tensor_tensor
(+accum); squares split between ScalarE activation(Square, accum) and DVE to
balance engine busy time. Small trailing chunks shrink the post-DMA straggle.
"""

import ml_dtypes
import numpy as np

import concourse.bacc as bacc
import concourse.tile as tile
from concourse import mybir
from concourse.bass_utils import run_bass_kernel_spmd

N, D = 32768, 1024
N_CORES = 8
ROWS_PER_CORE = N // N_CORES  # 4096
P = 128
CHUNKS = (128, 256, 512, 512, 512, 512, 512, 512, 384, 128, 128)  # rows per dma_start
MARGIN = 0.5
EPS = 1e-8

F32 = mybir.dt.float32
BF16 = mybir.dt.bfloat16
U8 = mybir.dt.uint8
Alu = mybir.AluOpType
Act = mybir.ActivationFunctionType

assert sum(CHUNKS) == ROWS_PER_CORE

# of the 32 yy squares, this many go to ACT (rest to DVE), evenly interleaved
ACT_YY = 10


def _perm(n_tiles=None):
    n = n_tiles or sum(R // P for R in CHUNKS)
    acts = [t for t in range(n) if (t * ACT_YY) // 32 != ((t + 1) * ACT_YY) // 32]
    dves = [t for t in range(n) if t not in acts]
    perm = [0] * n
    for i, t in enumerate(acts + dves):
        perm[t] = i
    return perm, len(acts)


def _col_row_map(chunks=CHUNKS):
    """col_rows[p, k] = local row index feeding stats column k at partition p."""
    n_cols = sum(R // P for R in chunks)
    perm, _ = _perm(n_cols)
    col_rows = np.empty((P, n_cols), dtype=np.int64)
    k = 0
    r0 = 0
    for R in chunks:
        s_count = R // P
        for s in range(s_count):
            col_rows[:, perm[k]] = r0 + np.arange(P) * s_count + s
            k += 1
        r0 += R
    return col_rows


def build(d=D, chunks=CHUNKS):
    n_tiles = sum(R // P for R in chunks)
    rows_per_core = sum(chunks)
    max_s = max(R // P for R in chunks)

    nc = bacc.Bacc(
        "TRN2",
        target_bir_lowering=False,
        debug=False,
        enable_asserts=False,
        num_devices=N_CORES,
    )
    xy_dram = nc.dram_tensor("xy", [2 * rows_per_core, d], BF16, kind="ExternalInput")
    m_dram = nc.dram_tensor("m", [P, n_tiles], U8, kind="ExternalInput")
    o_dram = nc.dram_tensor("out", [1, 1], F32, kind="ExternalOutput")

    with tile.TileContext(nc) as tc:
        with (
            tc.tile_pool(name="xyin", bufs=5) as xypool,
            tc.tile_pool(name="scratch", bufs=1) as spool,
            tc.tile_pool(name="stats", bufs=1) as statpool,
            tc.tile_pool(name="ep", bufs=1) as eppool,
            tc.tile_pool(name="psum", bufs=1, space="PSUM") as psumpool,
        ):
            dot_s = statpool.tile([P, n_tiles], F32)
            xx_s = statpool.tile([P, n_tiles], F32)
            n_act_yy0 = _perm(n_tiles)[1]
            yy_a = statpool.tile([P, n_act_yy0], F32)            # ACT yy cols
            yy_d = statpool.tile([P, n_tiles - n_act_yy0], F32)  # DVE yy cols
            mask_t = statpool.tile([P, n_tiles], U8)
            zero_t = statpool.tile([P, 1], F32)
            negm_t = statpool.tile([P, 1], F32)
            dummy_t = statpool.tile([P, 1], F32)
            ones_t = statpool.tile([P, 1], F32)
            # engine-private scratch outputs, reused across iterations
            prod_t = spool.tile([P, D], BF16)
            junk_act = spool.tile([P, D], BF16)
            junk_dve = spool.tile([P, D], BF16)
            nc.vector.memset(ones_t, 1.0)
            nc.vector.memset(zero_t, 0.0)
            nc.vector.memset(negm_t, -MARGIN)
            # First ACT op is a Sqrt so bacc loads the sqrt_and_others table
            # set once; Square/Relu/Copy/Identity are all in that set too.
            nc.scalar.activation(dummy_t, zero_t, Act.Sqrt, bias=zero_t)

            perm, n_act_yy = _perm(n_tiles)
            xyap = xy_dram.ap()
            r0 = 0
            t = 0
            ta = 0
            td = 0
            for R in chunks:
                s_count = R // P
                xy_t = xypool.tile([P, 2, max_s, d], BF16, tag="xy")
                nc.sync.dma_start(
                    out=xy_t[:, :, :s_count, :],
                    in_=xyap[2 * r0 : 2 * r0 + 2 * R, :].rearrange(
                        "(w p s) d -> p w s d", w=2, p=P
                    ),
                )
                for s in range(s_count):
                    pc = perm[t]
                    x_sl = xy_t[:, 0, s, :]
                    y_sl = xy_t[:, 1, s, :]
                    # dot on DVE
                    nc.vector.scalar_tensor_tensor(
                        out=prod_t,
                        in0=x_sl,
                        scalar=1.0,
                        in1=y_sl,
                        op0=Alu.mult,
                        op1=Alu.mult,
                        accum_out=dot_s[:, pc : pc + 1],
                    )
                    # xx on ACT (1x, dtype-independent)
                    nc.scalar.activation(
                        out=junk_act,
                        in_=x_sl,
                        func=Act.Square,
                        bias=zero_t,
                        accum_out=xx_s[:, pc : pc + 1],
                    )
                    # yy: split so DVE and ACT busy times balance
                    # (DVE op ~1.30us incl accum read, ACT ~1.43us;
                    #  DVE: 32 dots + 18 yy, ACT: 32 xx + 14 yy),
                    # interleaved so neither engine starves mid-stream
                    if (t * ACT_YY) // 32 == ((t + 1) * ACT_YY) // 32:
                        nc.vector.scalar_tensor_tensor(
                            out=junk_dve,
                            in0=y_sl,
                            scalar=1.0,
                            in1=y_sl,
                            op0=Alu.mult,
                            op1=Alu.mult,
                            accum_out=yy_d[:, td : td + 1],
                        )
                        td += 1
                    else:
                        nc.scalar.activation(
                            out=junk_act,
                            in_=y_sl,
                            func=Act.Square,
                            bias=zero_t,
                            accum_out=yy_a[:, ta : ta + 1],
                        )
                        ta += 1
                    t += 1
                r0 += R

            # mask is only needed by the epilogue; don't delay chunk DMAs
            nc.sync.dma_start(out=mask_t, in_=m_dram.ap())

            # ---- epilogue on (P, n_tiles) stats ----
            pr = eppool.tile([P, n_tiles], F32)
            nc.vector.tensor_mul(pr[:, :n_act_yy0], xx_s[:, :n_act_yy0], yy_a)
            nc.vector.tensor_mul(pr[:, n_act_yy0:], xx_s[:, n_act_yy0:], yy_d)
            s_ = eppool.tile([P, n_tiles], F32)
            nc.scalar.activation(s_, pr, Act.Sqrt, bias=zero_t)
            rs = eppool.tile([P, n_tiles], F32)
            nc.vector.reciprocal(rs, s_)
            dd = eppool.tile([P, n_tiles], F32)
            nc.vector.tensor_mul(dd, dot_s, rs)
            pos = eppool.tile([P, n_tiles], F32)  # 1 - d
            nc.scalar.activation(pos, dd, Act.Copy, bias=1.0, scale=-1.0)
            neg = eppool.tile([P, n_tiles], F32)  # relu(d - margin)
            nc.scalar.activation(neg, dd, Act.Relu, bias=negm_t)
            per = eppool.tile([P, n_tiles], F32)
            nc.vector.select(per, mask_t, pos, neg)
            row = eppool.tile([P, 1], F32)
            nc.vector.reduce_sum(row, per, axis=mybir.AxisListType.X)
            ps = psumpool.tile([1, 1], F32)
            nc.tensor.matmul(out=ps, lhsT=row, rhs=ones_t, start=True, stop=True)
            res = eppool.tile([1, 1], F32)
            nc.scalar.copy(res, ps)
            nc.sync.dma_start(out=o_dram.ap(), in_=res)

    nc.compile()
    return nc


_cached_nc = None


def _get_nc():
    global _cached_nc
    if _cached_nc is None:
        _cached_nc = build()
    return _cached_nc


def _interleave_xy(x_shard, y_shard, d, chunks=CHUNKS):
    rows = x_shard.shape[0]
    xy = np.empty((2 * rows, d), dtype=ml_dtypes.bfloat16)
    r0 = 0
    for R in chunks:
        xy[2 * r0 : 2 * r0 + R] = x_shard[r0 : r0 + R]
        xy[2 * r0 + R : 2 * r0 + 2 * R] = y_shard[r0 : r0 + R]
        r0 += R
    return xy


def _make_in_maps(x, y, p):
    x = np.asarray(x, dtype=np.float32)
    y = np.asarray(y, dtype=np.float32)
    m_full = (np.asarray(p) == 1).astype(np.uint8)
    col_rows = _col_row_map()
    in_maps = []
    for c in range(N_CORES):
        base = c * ROWS_PER_CORE
        sl = slice(base, base + ROWS_PER_CORE)
        in_maps.append(
            {
                "xy": _interleave_xy(x[sl], y[sl], D),
                "m": np.ascontiguousarray(m_full[base + col_rows]),
            }
        )
    return in_maps


def run(x, y, p, trace=False):
    """Returns (loss_scalar_f32, exec_time_ns_or_None)."""
    nc = _get_nc()
    in_maps = _make_in_maps(x, y, p)
    res = run_bass_kernel_spmd(nc, in_maps, list(range(N_CORES)), trace=trace)
    partials = np.array([r["out"][0, 0] for r in res.results], dtype=np.float32)
    total = np.float32(np.sum(partials, dtype=np.float32))
    return total, res.exec_time_ns


def kernel(x, y, p):
    total, _ = run(x, y, p)
    return total



# revision 15
# speedup vs baseline: 1.4312x; 1.4312x over previous
"""CosineEmbeddingLoss kernel for Trainium2 (Bass/Tile), 8-core data parallel.

reference semantics (fp32):
    dot   = sum(x*y, -1); xx = sum(x*x, -1); yy = sum(y*y, -1)
    d     = dot / max(sqrt(xx*yy), EPS)
    per   = where(p == 1, 1 - d, max(0, d - MARGIN))
    loss  = sum(per)

Strategy (per core, 4096 rows, all inputs host-cast to fp8e4m3 -> 8.4MB DMA):
 - B_PE blocks of 64 rows go through the TensorEngine as Gram matrices:
   T = [x_rows | y_rows] laid out d-on-partitions; G = T^T T computed with
   fp8 DoubleRow matmuls (contraction 256 per instruction, 4 per block)
   accumulated in PSUM. G's diagonal holds xx (cols 0:64) and yy (64:128),
   and G[i, 64+i] = dot_i. ACT copies PSUM->SBUF as bf16 (4 Grams/op);
   DVE extracts diag + off-diag via tensor_mask_reduce (op=max, window
   [p, p+1)) into per-block stat columns.
 - G_ENG groups of 128 rows stay row-on-partition: DVE scalar_tensor_tensor
   computes dot (and yy for YY_DVE groups), ACT activation(Square, accum)
   computes xx (and remaining yy). This fills DVE/ACT time not used by
   extraction so the kernel is DMA-bound.
 - Epilogue on tiny (64,B)/(128,G) stat tiles; per-core scalar out; host
   sums the 8 partials.
"""

import ml_dtypes
import numpy as np

import concourse.bacc as bacc
import concourse.tile as tile
from concourse import mybir
from concourse.bass_utils import run_bass_kernel_spmd

N, D = 32768, 1024
N_CORES = 8
RPC = N // N_CORES  # 4096 rows per core

B_PE = 52   # 64-row Gram blocks on the TensorEngine
G_ENG = 6   # 128-row elementwise groups on DVE/ACT
YY_DVE = 0  # of the G_ENG yy squares, this many go to DVE (rest ACT)
assert 64 * B_PE + 128 * G_ENG == RPC

# pe-stream dma chunk sizes in blocks (front-loaded small for early PE start)
PE_DMA_CHUNKS = (2, 3, 5, 6, 8, 8, 8, 6, 6)
assert sum(PE_DMA_CHUNKS) == B_PE

MARGIN = 0.5
EPS = 1e-8

F32 = mybir.dt.float32
BF16 = mybir.dt.bfloat16
FP8 = mybir.dt.float8e4
U8 = mybir.dt.uint8
Alu = mybir.AluOpType
Act = mybir.ActivationFunctionType

NP_FP8 = ml_dtypes.float8_e4m3


def build(num_devices=N_CORES):
    nc = bacc.Bacc(
        "TRN2",
        target_bir_lowering=False,
        debug=False,
        enable_asserts=False,
        num_devices=num_devices,
    )
    pe_dram = nc.dram_tensor("pe", [128, B_PE * 8, 128], FP8, kind="ExternalInput")
    eng_dram = nc.dram_tensor("eng", [128, 2 * G_ENG, 1024], FP8, kind="ExternalInput")
    mpe_dram = nc.dram_tensor("mpe", [64, B_PE], U8, kind="ExternalInput")
    me_dram = nc.dram_tensor("me", [128, G_ENG], U8, kind="ExternalInput")
    eye_dram = nc.dram_tensor("eye", [128, 128], BF16, kind="ExternalInput")
    o_dram = nc.dram_tensor("out", [1, 1], F32, kind="ExternalOutput")

    n_psg = (B_PE + 3) // 4  # psum groups of up to 4 Grams

    with tile.TileContext(nc) as tc:
        with (
            tc.tile_pool(name="big", bufs=1) as bigpool,
            tc.tile_pool(name="gcat", bufs=3) as gcatpool,
            tc.tile_pool(name="stat", bufs=1) as statpool,
            tc.tile_pool(name="psum", bufs=4, space="PSUM") as psumpool,
            tc.tile_pool(name="psc", bufs=1, space="PSUM") as pscpool,
        ):
            pe_all = bigpool.tile([128, B_PE * 8, 128], FP8)
            eng_all = bigpool.tile([128, 2 * G_ENG, 1024], FP8)

            diag_s = statpool.tile([128, B_PE], F32)  # xx (p<64) / yy (p>=64)
            dot_s = statpool.tile([64, B_PE], F32)
            dot_e = statpool.tile([128, G_ENG], F32)
            xx_e = statpool.tile([128, G_ENG], F32)
            yy_e = statpool.tile([128, G_ENG], F32)
            mpe_t = statpool.tile([64, B_PE], U8)
            me_t = statpool.tile([128, G_ENG], U8)
            eye_t = statpool.tile([128, 128], BF16)
            zero_t = statpool.tile([128, 1], F32)
            ones_t = statpool.tile([128, 1], F32)
            negm_t = statpool.tile([128, 1], F32)
            dummy_t = statpool.tile([128, 1], F32)
            # engine-private junk outputs, reused across iterations
            junk_mr = statpool.tile([128, 128], BF16)
            junk_mr64 = statpool.tile([64, 64], BF16)
            junk_dve = statpool.tile([128, 1024], BF16)
            junk_act = statpool.tile([128, 1024], BF16)

            nc.vector.memset(zero_t, 0.0)
            nc.vector.memset(ones_t, 1.0)
            nc.vector.memset(negm_t, -MARGIN)
            # first ACT op is a Sqrt so bacc loads the sqrt_and_others table
            # once; Square/Relu/Copy are all in that set too.
            nc.scalar.activation(dummy_t, zero_t, Act.Sqrt, bias=zero_t)

            # ---- input DMAs (small tensors first, then bulk streams) ----
            nc.sync.dma_start(out=eye_t, in_=eye_dram.ap())
            nc.sync.dma_start(out=mpe_t, in_=mpe_dram.ap())
            nc.sync.dma_start(out=me_t, in_=me_dram.ap())
            peap = pe_dram.ap()
            b0 = 0
            for nb in PE_DMA_CHUNKS:
                nc.sync.dma_start(
                    out=pe_all[:, 8 * b0 : 8 * (b0 + nb), :],
                    in_=peap[:, 8 * b0 : 8 * (b0 + nb), :],
                )
                b0 += nb
            engap = eng_dram.ap()
            for g in range(G_ENG):
                nc.sync.dma_start(
                    out=eng_all[:, 2 * g : 2 * g + 2, :],
                    in_=engap[:, 2 * g : 2 * g + 2, :],
                )

            def eng_group(g):
                xap = eng_all[:, 2 * g, :]
                yap = eng_all[:, 2 * g + 1, :]
                nc.vector.scalar_tensor_tensor(
                    out=junk_dve,
                    in0=xap,
                    scalar=1.0,
                    in1=yap,
                    op0=Alu.mult,
                    op1=Alu.mult,
                    accum_out=dot_e[:, g : g + 1],
                )
                nc.scalar.activation(
                    out=junk_act,
                    in_=xap,
                    func=Act.Square,
                    bias=zero_t,
                    accum_out=xx_e[:, g : g + 1],
                )
                if g < YY_DVE:
                    nc.vector.scalar_tensor_tensor(
                        out=junk_dve,
                        in0=yap,
                        scalar=1.0,
                        in1=yap,
                        op0=Alu.mult,
                        op1=Alu.mult,
                        accum_out=yy_e[:, g : g + 1],
                    )
                else:
                    nc.scalar.activation(
                        out=junk_act,
                        in_=yap,
                        func=Act.Square,
                        bias=zero_t,
                        accum_out=yy_e[:, g : g + 1],
                    )

            # ---- main loop: psum groups of 4 Grams ----
            g_emitted = 0
            for q in range(n_psg):
                blocks = range(4 * q, min(4 * q + 4, B_PE))
                ps = psumpool.tile([128, 512], F32, tag="ps")
                for k, b in enumerate(blocks):
                    for c in range(4):
                        ap = pe_all[:, 8 * b + 2 * c : 8 * b + 2 * c + 2, :]
                        nc.tensor.matmul(
                            out=ps[:, 128 * k : 128 * (k + 1)],
                            lhsT=ap,
                            rhs=ap,
                            start=(c == 0),
                            stop=(c == 3),
                            perf_mode=mybir.MatmulPerfMode.DoubleRow,
                        )
                nk = len(blocks)
                gc = gcatpool.tile([128, 512], BF16, tag="gc")
                nc.scalar.copy(gc[:, : 128 * nk], ps[:, : 128 * nk])
                for k, b in enumerate(blocks):
                    # diag(G) = [xx; yy] via masked multiply-accumulate
                    nc.vector.scalar_tensor_tensor(
                        out=junk_mr,
                        in0=gc[:, 128 * k : 128 * (k + 1)],
                        scalar=1.0,
                        in1=eye_t,
                        op0=Alu.mult,
                        op1=Alu.mult,
                        accum_out=diag_s[:, b : b + 1],
                    )
                    # diag of top-right quadrant = dot
                    nc.vector.scalar_tensor_tensor(
                        out=junk_mr64,
                        in0=gc[0:64, 128 * k + 64 : 128 * (k + 1)],
                        scalar=1.0,
                        in1=eye_t[0:64, 0:64],
                        op0=Alu.mult,
                        op1=Alu.mult,
                        accum_out=dot_s[:, b : b + 1],
                    )
                # spread engine-group work between psum groups
                if q % 2 == 1 and g_emitted < G_ENG:
                    eng_group(g_emitted)
                    g_emitted += 1
            while g_emitted < G_ENG:
                eng_group(g_emitted)
                g_emitted += 1

            # ---- epilogue: PE-side stats (64, B_PE) ----
            ep = statpool
            yy_t = ep.tile([64, B_PE], F32)
            # move yy from partitions 64:128 down to 0:64 (partition-crossing DMA)
            nc.sync.dma_start(out=yy_t, in_=diag_s[64:128, :])
            prodp = ep.tile([64, B_PE], F32)
            nc.vector.tensor_mul(prodp, diag_s[0:64, :], yy_t)
            sp = ep.tile([64, B_PE], F32)
            nc.scalar.activation(sp, prodp, Act.Sqrt, bias=0.0)
            nc.vector.tensor_scalar_max(sp, sp, EPS)
            rp = ep.tile([64, B_PE], F32)
            nc.vector.reciprocal(rp, sp)
            dpe = ep.tile([64, B_PE], F32)
            nc.vector.tensor_mul(dpe, dot_s, rp)
            posp = ep.tile([64, B_PE], F32)
            nc.scalar.activation(posp, dpe, Act.Copy, bias=1.0, scale=-1.0)
            negp = ep.tile([64, B_PE], F32)
            nc.scalar.activation(negp, dpe, Act.Relu, bias=negm_t[0:64, :])
            perp = ep.tile([64, B_PE], F32)
            nc.vector.select(perp, mpe_t, posp, negp)
            r1 = ep.tile([64, 1], F32)
            nc.vector.reduce_sum(r1, perp, axis=mybir.AxisListType.X)

            # ---- epilogue: engine-side stats (128, G_ENG) ----
            prode = ep.tile([128, G_ENG], F32)
            nc.vector.tensor_mul(prode, xx_e, yy_e)
            se = ep.tile([128, G_ENG], F32)
            nc.scalar.activation(se, prode, Act.Sqrt, bias=0.0)
            nc.vector.tensor_scalar_max(se, se, EPS)
            re_ = ep.tile([128, G_ENG], F32)
            nc.vector.reciprocal(re_, se)
            de = ep.tile([128, G_ENG], F32)
            nc.vector.tensor_mul(de, dot_e, re_)
            pose = ep.tile([128, G_ENG], F32)
            nc.scalar.activation(pose, de, Act.Copy, bias=1.0, scale=-1.0)
            nege = ep.tile([128, G_ENG], F32)
            nc.scalar.activation(nege, de, Act.Relu, bias=negm_t)
            pere = ep.tile([128, G_ENG], F32)
            nc.vector.select(pere, me_t, pose, nege)
            r2 = ep.tile([128, 1], F32)
            nc.vector.reduce_sum(r2, pere, axis=mybir.AxisListType.X)

            # ---- total = (r2 with r1 added into its low half) . ones ----
            racc = ep.tile([128, 1], F32)
            nc.vector.tensor_add(racc[0:64, :], r2[0:64, :], r1)
            nc.vector.tensor_copy(out=racc[64:128, :], in_=r2[64:128, :])
            ps1 = pscpool.tile([1, 1], F32)
            nc.tensor.matmul(out=ps1, lhsT=racc, rhs=ones_t, start=True, stop=True)
            res = ep.tile([1, 1], F32)
            nc.scalar.copy(res, ps1)
            nc.sync.dma_start(out=o_dram.ap(), in_=res)

    nc.compile()
    return nc


_cached_nc = None


def _get_nc():
    global _cached_nc
    if _cached_nc is None:
        _cached_nc = build()
    return _cached_nc


def make_core_inputs(x8, y8, m, core):
    """Pack one core's inputs. x8/y8: (N, D) fp8 arrays; m: (N,) uint8."""
    base = core * RPC
    pe_rows = 64 * B_PE
    xr = x8[base : base + pe_rows].reshape(B_PE, 64, 4, 2, 128)
    yr = y8[base : base + pe_rows].reshape(B_PE, 64, 4, 2, 128)
    # [b, i, c, t, p] -> [b, c, t, p, i]
    xt = xr.transpose(0, 2, 3, 4, 1)
    yt = yr.transpose(0, 2, 3, 4, 1)
    arr = np.concatenate([xt, yt], axis=4)  # [b, c, t, p, 128]
    pe_host = np.ascontiguousarray(
        arr.transpose(3, 0, 1, 2, 4).reshape(128, B_PE * 8, 128)
    )
    ex = x8[base + pe_rows : base + RPC].reshape(G_ENG, 128, 1024)
    ey = y8[base + pe_rows : base + RPC].reshape(G_ENG, 128, 1024)
    es = np.stack([ex, ey], axis=1)  # [g, s, p, d]
    eng_host = np.ascontiguousarray(es.transpose(2, 0, 1, 3).reshape(128, 2 * G_ENG, 1024))
    mpe_host = np.ascontiguousarray(m[base : base + pe_rows].reshape(B_PE, 64).T)
    me_host = np.ascontiguousarray(m[base + pe_rows : base + RPC].reshape(G_ENG, 128).T)
    return {
        "pe": pe_host,
        "eng": eng_host,
        "mpe": mpe_host,
        "me": me_host,
        "eye": np.eye(128, dtype=ml_dtypes.bfloat16),
    }


def _make_in_maps(x, y, p):
    x8 = np.asarray(x, dtype=np.float32).astype(NP_FP8)
    y8 = np.asarray(y, dtype=np.float32).astype(NP_FP8)
    m = (np.asarray(p) == 1).astype(np.uint8)
    return [make_core_inputs(x8, y8, m, c) for c in range(N_CORES)]


def run(x, y, p, trace=False):
    """Returns (loss_scalar_f32, exec_time_ns_or_None)."""
    nc = _get_nc()
    in_maps = _make_in_maps(x, y, p)
    res = run_bass_kernel_spmd(nc, in_maps, list(range(N_CORES)), trace=trace)
    partials = np.array([r["out"][0, 0] for r in res.results], dtype=np.float32)
    total = np.float32(np.sum(partials, dtype=np.float32))
    return total, res.exec_time_ns


def kernel(x, y, p):
    total, _ = run(x, y, p)
    return total
